# revision 10
# baseline (speedup 1.0000x reference)
"""Trainium2 Bass kernel for nn_CombinedTargetIOULoss (B=64, K=17, H=W=64).

Data-parallel over batch: 8 cores x 8 batches each. Each core computes
per-(b,k) partial sums [sum(q1+q2), sum((hp-hg)^2)] over the 4096 pixels;
the host combines them into the scalar loss (incl. target-weight scaling
and the tw==0 mask case).

Key algebra (the pixel anchors xs/ys cancel out of the reference box math):
  iw = (|p|+|g|-|p-g|)/2, cw = (|p|+|g|+|p-g|)/2  (same for y with q,h)
  inter = iw*ih, area_c = cw*ch, area_p = |p||q|, area_g = |g||h|
  union = area_p + area_g - inter + EPS
  giou_loss = 2 - inter/union - union/(area_c + EPS) = 2 - q1 - q2

SBUF layout: two batches stacked per tile, partition = (b%2)*64 + hx,
free = (ch=3k+c, hy). Every engine op covers all 128 partitions with a
uniform [128, (17,192),(64,1)] access pattern; per-(b,k) sums are done by
TensorE with one-hot stationary columns (psum row = local batch index).

Raw bass (no Tile): this walrus build rejects instructions carrying more
than one sem-wait, so all cross-engine sync is standalone wait_ge ops with
monotone per-engine counters.
"""

import sys

sys.path.insert(0, "/opt/trn_rl_repo")

import numpy as np

import concourse.bass as bass
from concourse import mybir
from concourse.alu_op_type import AluOpType as Alu
from concourse.bass_utils import run_bass_kernel_spmd

F32 = mybir.dt.float32
F16 = mybir.dt.float16
AF = mybir.ActivationFunctionType

EPS = 1e-7
B, K, H, W = 64, 17, 64, 64
C = 3 * K
P = H * W
N_CORES = 8
B_LOC = B // N_CORES
N_PAIR = B_LOC // 2

J = 64            # hy run (256B DMA descriptors)
MIDF = K * J      # 1088: free size of per-component intermediate tiles
INF = C * J       # 3264: free size of input tiles
# matmul column splits, k-aligned, each <= 512 cols and <= 1 PSUM bank
SPLITS = [(0, 6), (6, 6), (12, 5)]

N_DVE = 20        # DVE ops per pair-iteration
N_ACT = 8         # ACT ops per pair-iteration
N_PE = 6          # matmuls per pair-iteration


class _Waiter:
    """Dedupe monotone standalone waits per (engine, sem)."""

    def __init__(self):
        self.seen = {}

    def wait(self, eng, sem, val):
        key = (id(eng), sem.name if hasattr(sem, "name") else id(sem))
        if self.seen.get(key, -1) >= val:
            return
        self.seen[key] = val
        eng.wait_ge(sem, val)


def _build_body(nc, o_ext, t_ext, p_ext, repeat=1):
    sb = lambda name, shape, dt: nc.alloc_sbuf_tensor(name, shape, dt).ap()

    # --- memory ---
    to = [sb(f"to{s}", [128, INF], F32) for s in range(2)]
    tt = [sb(f"tt{s}", [128, INF], F32) for s in range(2)]
    mids = {}
    for nm in ("ap ag aq ah dx dy ex ey d sx sy u2 uy2 v2 vy2 t1 t2 it4 ac4 "
               "s ue rcu rcc ace q1 q2").split():
        mids[nm] = sb(nm, [128, MIDF], F32)
    qs = sb("qs", [128, MIDF], F16)
    dsq = sb("dsq", [128, MIDF], F16)
    wts = [sb(f"w{j}", [128, B_LOC], F16) for j in range(N_PAIR)]
    osb = sb("osb", [B_LOC, 2 * K], F32)
    dmy = sb("dmy", [128, 4], F32)
    ps = []
    for qi in range(2):
        for si, (k0, n) in enumerate(SPLITS):
            ps.append(nc.alloc_psum_tensor(f"ps{qi}{si}", [B_LOC, n * J], F32).ap())

    # --- semaphores ---
    dma_in = nc.alloc_semaphore("dma_in")
    dma_out = nc.alloc_semaphore("dma_out")
    act_c = nc.alloc_semaphore("act_c")
    dve_c = nc.alloc_semaphore("dve_c")
    pe_c = nc.alloc_semaphore("pe_c")
    gp_c = nc.alloc_semaphore("gp_c")
    wt = _Waiter()

    # --- warmup: absorb ACT table loads on dependency-free instructions ---
    nc.scalar.activation(dmy[:, 0:1], dmy[:, 3:4], AF.Abs)
    nc.scalar.activation(dmy[:, 1:2], dmy[:, 3:4], AF.Square)
    nc.scalar.activation(dmy[:, 2:3], dmy[:, 3:4], AF.Copy, bias=0.0, scale=1.0)

    # --- one-hot stationary weights (GPSIMD) ---
    gp_n = 0
    for j in range(N_PAIR):
        nc.gpsimd.memset(wts[j][:], 0.0).then_inc(gp_c, 1)
        nc.gpsimd.memset(wts[j][0:64, 2 * j : 2 * j + 1], 1.0).then_inc(gp_c, 1)
        nc.gpsimd.memset(wts[j][64:128, 2 * j + 1 : 2 * j + 2], 1.0).then_inc(gp_c, 1)
        gp_n += 3

    def act(pos_ignored, out, in_, func, **kw):
        nc.scalar.activation(out, in_, func, **kw).then_inc(act_c, 1)

    def dve_tt(out, a, b, op):
        nc.vector.tensor_tensor(out, a, b, op).then_inc(dve_c, 1)

    def comp(T, c):
        return T.rearrange("p (k c hy) -> p k c hy", k=K, c=3, hy=J)[:, :, c]

    m = lambda nm: mids[nm][:]

    n_iter = repeat * N_PAIR
    for j in range(n_iter):
        jp = j % N_PAIR       # which batch pair (repeat>1 reruns for timing)
        sl = j % 2
        dve0 = N_DVE * j      # dve count before this iter
        act0 = N_ACT * j
        pe0 = N_PE * j

        # --- DMA in (SP): WAR vs readers from iteration j-2 (same slot) ---
        if j >= 2:
            wt.wait(nc.sync, dve_c, N_DVE * (j - 2) + 3)   # d read to/tt
            wt.wait(nc.sync, act_c, N_ACT * (j - 2) + 4)   # ah read tt
        for src, T in ((o_ext, to[sl]), (t_ext, tt[sl])):
            for pi in range(2):
                nc.sync.dma_start(
                    out=T[64 * pi : 64 * pi + 64, :].rearrange(
                        "p (ch hy) -> p ch hy", ch=C, hy=J
                    ),
                    in_=src[2 * jp + pi].rearrange("ch hx hy -> hx ch hy"),
                ).then_inc(dma_in, 16)

        # --- ACT stream ---
        if j >= 1:
            # overwrite guard: previous iter's DVE consumers of ap..ah
            wt.wait(nc.scalar, dve_c, N_DVE * (j - 1) + 11)  # t2 done
        wt.wait(nc.scalar, dma_in, 64 * j + 32)
        act(1, m("ap"), comp(to[sl], 1), AF.Abs)
        wt.wait(nc.scalar, dma_in, 64 * j + 64)
        act(2, m("ag"), comp(tt[sl], 1), AF.Abs)
        act(3, m("aq"), comp(to[sl], 2), AF.Abs)
        act(4, m("ah"), comp(tt[sl], 2), AF.Abs)
        wt.wait(nc.scalar, dve_c, dve0 + 1)
        act(5, m("dx"), m("ex"), AF.Abs)
        wt.wait(nc.scalar, dve_c, dve0 + 2)
        act(6, m("dy"), m("ey"), AF.Abs)
        if j >= 1:
            wt.wait(nc.scalar, pe_c, N_PE * (j - 1) + 6)   # dsq consumed by PE
        wt.wait(nc.scalar, dve_c, dve0 + 3)
        act(7, dsq[:], m("d"), AF.Square)
        if j >= 1:
            wt.wait(nc.scalar, dve_c, N_DVE * (j - 1) + 17)  # rcc read ace
        wt.wait(nc.scalar, dve_c, dve0 + 13)
        act(8, m("ace"), m("ac4"), AF.Copy, bias=EPS, scale=0.25)

        # --- DVE stream ---
        if j >= 1:
            wt.wait(nc.vector, act_c, N_ACT * (j - 1) + 8)   # ACT j-1 fully done
        wt.wait(nc.vector, dma_in, 64 * j + 64)
        dve_tt(m("ex"), comp(to[sl], 1), comp(tt[sl], 1), Alu.subtract)   # 1
        dve_tt(m("ey"), comp(to[sl], 2), comp(tt[sl], 2), Alu.subtract)   # 2
        dve_tt(m("d"), comp(to[sl], 0), comp(tt[sl], 0), Alu.subtract)    # 3
        wt.wait(nc.vector, act_c, act0 + 2)
        dve_tt(m("sx"), m("ap"), m("ag"), Alu.add)                        # 4
        wt.wait(nc.vector, act_c, act0 + 4)
        dve_tt(m("sy"), m("aq"), m("ah"), Alu.add)                        # 5
        wt.wait(nc.vector, act_c, act0 + 5)
        dve_tt(m("u2"), m("sx"), m("dx"), Alu.subtract)                   # 6
        wt.wait(nc.vector, act_c, act0 + 6)
        dve_tt(m("uy2"), m("sy"), m("dy"), Alu.subtract)                  # 7
        dve_tt(m("v2"), m("sx"), m("dx"), Alu.add)                        # 8
        dve_tt(m("vy2"), m("sy"), m("dy"), Alu.add)                       # 9
        dve_tt(m("t1"), m("ap"), m("aq"), Alu.mult)                       # 10
        dve_tt(m("t2"), m("ag"), m("ah"), Alu.mult)                       # 11
        dve_tt(m("it4"), m("u2"), m("uy2"), Alu.mult)                     # 12
        dve_tt(m("ac4"), m("v2"), m("vy2"), Alu.mult)                     # 13
        nc.vector.scalar_tensor_tensor(
            m("s"), m("t1"), EPS, m("t2"), Alu.add, Alu.add
        ).then_inc(dve_c, 1)                                              # 14
        nc.vector.scalar_tensor_tensor(
            m("ue"), m("it4"), -0.25, m("s"), Alu.mult, Alu.add
        ).then_inc(dve_c, 1)                                              # 15
        nc.vector.reciprocal_approx_fast(m("rcu"), m("ue")).then_inc(dve_c, 1)  # 16
        wt.wait(nc.vector, act_c, act0 + 8)
        nc.vector.reciprocal_approx_fast(m("rcc"), m("ace")).then_inc(dve_c, 1)  # 17
        nc.vector.scalar_tensor_tensor(
            m("q1"), m("it4"), 0.25, m("rcu"), Alu.mult, Alu.mult
        ).then_inc(dve_c, 1)                                              # 18
        dve_tt(m("q2"), m("ue"), m("rcc"), Alu.mult)                      # 19
        if j >= 1:
            wt.wait(nc.vector, pe_c, N_PE * (j - 1) + 3)   # qs consumed by PE
        dve_tt(qs[:], m("q1"), m("q2"), Alu.add)                          # 20

        # --- PE stream: per-(b,k) pixel sums ---
        if j == 0:
            wt.wait(nc.tensor, gp_c, gp_n)
        for qi, qt in enumerate((qs, dsq)):
            if qi == 0:
                wt.wait(nc.tensor, dve_c, dve0 + 20)
            else:
                wt.wait(nc.tensor, act_c, act0 + 7)
            for si, (k0, n) in enumerate(SPLITS):
                nc.tensor.matmul(
                    ps[qi * 3 + si][:],
                    wts[jp][:],
                    qt[:, k0 * J : (k0 + n) * J],
                    start=(j == 0),
                    stop=(j == n_iter - 1),
                ).then_inc(pe_c, 1)

    # --- epilogue: reduce hy columns on DVE, then store ---
    wt.wait(nc.vector, pe_c, N_PE * n_iter)
    nred = 0
    for qi in range(2):
        for si, (k0, n) in enumerate(SPLITS):
            pv = ps[qi * 3 + si].rearrange("p (k hy) -> p k hy", k=n, hy=J)
            nc.vector.tensor_reduce(
                osb[:, qi * K + k0 : qi * K + k0 + n],
                pv,
                mybir.AxisListType.X,
                Alu.add,
            ).then_inc(dve_c, 1)
            nred += 1
    wt.wait(nc.sync, dve_c, N_DVE * n_iter + nred)
    nc.sync.dma_start(out=p_ext[:], in_=osb[:]).then_inc(dma_out, 16)
    nc.sync.wait_ge(dma_out, 16)


def build_nc(repeat=1):
    nc = bass.Bass()
    o_ext = nc.declare_dram_parameter("output", [B_LOC, C, H, W], F32, isOutput=False)
    t_ext = nc.declare_dram_parameter("target", [B_LOC, C, H, W], F32, isOutput=False)
    p_ext = nc.declare_dram_parameter("partials", [B_LOC, 2 * K], F32, isOutput=True)
    _build_body(nc, o_ext, t_ext, p_ext, repeat=repeat)
    # fill the 64-byte ISA encodings of custom DVE ops (reciprocal_approx):
    # Bacc.compile() does this; the raw-Bass + PJRT path does not.
    mybir.codegen_inst_isa_subclasses(nc)
    return nc


_NC = None


def _get_nc():
    global _NC
    if _NC is None:
        _NC = build_nc()
    return _NC


def _combine(parts, target_weights):
    """parts: [8 cores, 8, 34] f32 -> scalar loss (host-side finish)."""
    arr = np.asarray(parts, np.float64).reshape(B, 2 * K)
    sqs = arr[:, :K]        # sum over pixels of (q1 + q2), per (b, k)
    ssd = arr[:, K:]        # sum over pixels of (hp - hg)^2, per (b, k)

    tw = np.asarray(target_weights, np.float64)
    twnz = (tw != 0).astype(np.float64)
    num = ((2.0 * P - sqs) * twnz).sum(axis=0)
    den = np.maximum((P * twnz).sum(axis=0), 1.0)
    giou_joint = num / den
    mse = 0.5 * (tw**2 * ssd).sum(axis=0) / (B * P)
    return np.float32(np.sum(mse + giou_joint) / K)


def kernel(output, target, target_weights):
    output = np.ascontiguousarray(np.asarray(output), dtype=np.float32)
    target = np.ascontiguousarray(np.asarray(target), dtype=np.float32)
    nc = _get_nc()
    in_maps = [
        {
            "output": output[i * B_LOC : (i + 1) * B_LOC],
            "target": target[i * B_LOC : (i + 1) * B_LOC],
        }
        for i in range(N_CORES)
    ]
    res = run_bass_kernel_spmd(nc, in_maps, list(range(N_CORES)))
    parts = np.stack([res.results[i]["partials"] for i in range(N_CORES)])
    return np.asarray(_combine(parts, target_weights), dtype=np.float32)
